# revision 56
# baseline (speedup 1.0000x reference)
"""DCT-attention Trainium2 kernel (8 NeuronCores, data-parallel over batch).

Reference math (per b, h):
    Qd = dct @ (Q*s);  Kd = dct @ (K*s*mask);  Vd = dct @ (V*mask)   # [M,D]
    E  = Qd @ Kd^T;  P = softmax(E, axis=-1);  ctx = P @ Vd          # [M,D]
    x  = dct^T @ ctx                                                 # [N,D]
with B,H,N,D = 8,12,2048,64, M = 256, s = D**-0.25.

Sharding: batch b -> core b (8 cores). Host folds scale into Q/K and mask into
K/V, transposes to [N, H*D], bf16-casts; matmuls run bf16 -> fp32 PSUM; output
returns bf16 and is cast to f32 on the host.

DCT parity symmetry: dct[k, N-1-i] = (-1)^k dct[k, i].  The host uploads X
folded as [A; B] with A = X[:N/2] + reverse(X[N/2:]), B = X[:N/2] -
reverse(X[N/2:]), and the M axis globally reordered to [even k | odd k].
Every projection then contracts over N/2 instead of N.

Differences vs the first working version (67.0us):
  * dct (inverse basis, 0.5MB) is no longer uploaded; it is built on-chip by
    16 PE transposes of dctT blocks during the PE bubble while k streams in.
    A negated copy of the odd-k rows (dctn) is evicted in the same window so
    the upper output half accumulates E-O without any back-half negate ops.
  * energies are emitted per HEAD-PAIR into one PSUM bank ([128,2,256]) and
    exp'd in a single ACT op per pair -> the serial exp chain halves
    (24 x ~480ns -> 12 x ~650ns).
  * phase B uses one PSUM tile per head for both m-blocks -> one reciprocal
    per head instead of two.
  * out-DMA issue alternates Sync (HWDGE) and GpSimd (SWDGE) queues; GpSimd
    cannot touch PSUM on TRN2, so descriptor generation is the only useful
    back-half work for it.
  * evictions/normalize split across Vector/Scalar so neither chain gates
    the PE.
"""

import numpy as np
import ml_dtypes

B, H, N, D = 8, 12, 2048, 64
M = 256
HD = H * D          # 768
NH = N // 2         # 1024 folded length
NCH = NH // 128     # 8 folded chunks per parity phase
QT = 4              # chunks per input-DMA quarter (of the folded [N, HD] array)
MB = M // 128       # 2 m-blocks (even ks | odd ks)
HP = H // 2         # 6 head-pairs
FSPLIT = 2          # HD split for <=512-wide psum
FW = HD // FSPLIT   # 384
VW = D + 1          # 65: Vd columns + ones column

_BF16 = ml_dtypes.bfloat16
_CACHE = {}


def build_nc():
    import concourse.bacc as bacc
    import concourse.mybir as mybir
    import concourse.tile as tile
    from concourse.masks import make_identity
    from contextlib import ExitStack

    BF = mybir.dt.bfloat16
    F32 = mybir.dt.float32
    EXP = mybir.ActivationFunctionType.Exp
    COPY = mybir.ActivationFunctionType.Copy

    nc = bacc.Bacc()
    q_d = nc.declare_dram_parameter("q", [N, HD], BF, isOutput=False)
    k_d = nc.declare_dram_parameter("k", [N, HD], BF, isOutput=False)
    # v is uploaded fs-major ([fs, p, c, FW]) so the head-0-5 column half
    # arrives first: phase B h0-5 and the fs0 inverse DCT then overlap the
    # tail of the v stream.
    v_d = nc.declare_dram_parameter("v", [2 * N, FW], BF, isOutput=False)
    # dctT: [n' < N/2, m] with columns [even k | odd k]
    dctT_d = nc.declare_dram_parameter("dctT", [NH, M], BF, isOutput=False)
    out_d = nc.declare_dram_parameter("out", [2 * N, FW], BF, isOutput=True)

    # DRAM layouts are pre-tiled on the host to [partition, chunk, free]
    # so every DMA descriptor covers a 6KB (q/k/v) / 4KB (dctT) / 3KB (out)
    # contiguous run instead of 1.5KB rows scattered by the partition
    # interleave -- fewer descriptors, better sustained HBM rate.
    q_r = q_d.ap().rearrange("(p c) f -> p c f", c=2 * NCH)
    k_r = k_d.ap().rearrange("(p c) f -> p c f", c=2 * NCH)
    v_r = v_d.ap().rearrange("(f p c) w -> f p c w", f=FSPLIT, c=2 * NCH)
    dctT_r = dctT_d.ap().rearrange("(p c) m -> p c m", c=NCH)
    # out: [fs, p, c, FW] flattened to [N, HD] -- fs-major halves
    out_r = out_d.ap().rearrange("(f p c) w -> f p c w", f=FSPLIT, c=2 * NCH)

    with ExitStack() as ctx:
        tc = ctx.enter_context(tile.TileContext(nc))
        consts = ctx.enter_context(tc.tile_pool(name="consts", bufs=1))
        xin = ctx.enter_context(tc.tile_pool(name="xin", bufs=1))
        proj = ctx.enter_context(tc.tile_pool(name="proj", bufs=1))
        pbuf = ctx.enter_context(tc.tile_pool(name="pbuf", bufs=1))
        rbuf = ctx.enter_context(tc.tile_pool(name="rbuf", bufs=8))
        ostage = ctx.enter_context(tc.tile_pool(name="ostage", bufs=1))
        psA = ctx.enter_context(tc.tile_pool(name="psA", bufs=5, space="PSUM"))
        psE = ctx.enter_context(tc.tile_pool(name="psE", bufs=3, space="PSUM"))

        # ---- on-chip constants (no DMA) -----------------------------------
        ident = consts.tile([128, 128], BF)
        make_identity(nc, ident)
        ebias = consts.tile([128, 1], F32)
        nc.gpsimd.memset(ebias[:], -4.0)

        # ---- DMA stream (single FIFO queue; order == consumption order) ----
        # dctT split in two so Q-proj chunk 0 can start half a transfer early
        dctT_sb = consts.tile([128, NCH, M], BF)       # [n'-part, chunk, m]
        nc.sync.dma_start(dctT_sb[:, 0:NCH // 2, :], dctT_r[:, 0:NCH // 2, :])
        nc.sync.dma_start(dctT_sb[:, NCH // 2:, :], dctT_r[:, NCH // 2:, :])

        def quarters(name, src_r):
            tiles = []
            for qt in range(2 * NCH // QT):
                t = xin.tile([128, QT, HD], BF, tag=f"{name}{qt}")
                cs = slice(qt * QT, (qt + 1) * QT)
                nc.sync.dma_start(t[:], src_r[:, cs, :])
                tiles.append(t)
            return tiles

        q_t = quarters("q", q_r)   # chunks 0..7 = A-fold, 8..15 = B-fold
        k_t = quarters("k", k_r)
        # v: fs-major quarters ([128, QT, FW] each), fs0 then fs1
        v_t = [[], []]
        for fs in range(FSPLIT):
            for qt in range(2 * NCH // QT):
                t = xin.tile([128, QT, FW], BF, tag=f"v{fs}{qt}")
                cs = slice(qt * QT, (qt + 1) * QT)
                nc.sync.dma_start(t[:], v_r[fs, :, cs, :])
                v_t[fs].append(t)

        # ---- persistent intermediates ----
        # bdq: block-diagonal qdT per head-pair: [[qdT_h0, 0], [0, qdT_h1]]
        # ([128 d-pair rows, (head, m) cols]).  One full-128-partition matmul
        # against kdT then yields BOTH heads' energies side by side in one
        # PSUM bank, and one 512-wide exp covers the pair -> the serial ACT
        # exp chain halves (24 ops -> 12).
        bdq = [
            proj.tile([128, 2, M], BF, tag=f"bdq{hp}", name=f"bdq{hp}")
            for hp in range(HP)
        ]
        for hp in range(HP):
            nc.gpsimd.memset(bdq[hp][:], 0.0)
        kdT_sb = proj.tile([128, HP, M], BF, tag="kdT")
        vd_sb = proj.tile([128, MB, H, VW], BF, tag="vd")  # [m-part, mb, h, d+1]
        ctx_sb = proj.tile([128, MB, HD], BF, tag="ctx")   # [m-part, mb, h*d]
        # dct: [m-part, m-block, n'] built on-chip from dctT via PE transpose
        dct_sb = consts.tile([128, MB, NH], BF)
        nc.vector.memset(vd_sb[:, :, :, D:VW], 1.0)

        def xc(tiles, c):  # folded chunk c (0..15), [128, HD]
            return tiles[c // QT][:, c % QT, :]

        # eviction engine rotation (psum -> sbuf copies; GpSimd cannot
        # read PSUM on TRN2, so only Vector and Scalar qualify)
        def make_rot(*engines):
            state = {"i": 0}

            def rot(dst, src):
                e = engines[state["i"] % len(engines)]
                state["i"] += 1
                if e == "v":
                    nc.vector.tensor_copy(dst, src)
                else:
                    nc.scalar.activation(dst, src, COPY)
            return rot

        # ---- build dct (inverse basis) by transposing dctT blocks ---------
        # first thing on the PE: only needs dctT, runs while q streams in.
        # odd-k rows (mbb=1) are additionally evicted negated into dctn_sb
        # for the E-O accumulation of the reconstructed upper output half.
        dctn_sb = consts.tile([128, NH], BF)
        for c in range(NCH):
            for mbb in range(MB):
                pt = psA.tile([128, 128], BF, tag="A", name=f"t{c}{mbb}")
                nc.tensor.transpose(
                    pt[:], dctT_sb[:, c, mbb * 128:(mbb + 1) * 128], ident[:]
                )
                # balance the evict chain across both engines so the 5-deep
                # psA rotation never stalls the transpose stream
                if (c + mbb) % 2 == 0:
                    nc.vector.tensor_copy(
                        dct_sb[:, mbb, c * 128:(c + 1) * 128], pt[:]
                    )
                else:
                    nc.scalar.activation(
                        dct_sb[:, mbb, c * 128:(c + 1) * 128], pt[:], COPY
                    )
                if mbb == 1:
                    if c % 2 == 0:
                        nc.vector.tensor_scalar_mul(
                            dctn_sb[:, c * 128:(c + 1) * 128], pt[:], -1.0
                        )
                    else:
                        nc.scalar.activation(
                            dctn_sb[:, c * 128:(c + 1) * 128], pt[:], COPY,
                            scale=-1.0,
                        )

        # ---- Q/K projections ----------------------------------------------
        # Each parity phase runs as TWO chunk-major passes over 3 head-pair
        # groups (instead of one pass over 6):  the second pass refills the
        # chase bubbles left by the DMA pacing, frees a PSUM bank (psA=5,
        # psE=3 -> the exp chain never waits on the psE ping-pong), and for
        # K lets the first head-pairs evict one pass earlier, which starts
        # the serial exp chain sooner.
        def proj_pass(tiles, par, hps, tagn, after_chunk=None):
            groups = {
                hp: psA.tile([128, 128], F32, tag="A", name=f"g{tagn}{hp}")
                for hp in hps
            }
            for c in range(NCH):
                for hp in hps:
                    nc.tensor.matmul(
                        groups[hp][:],
                        lhsT=xc(tiles, par * NCH + c)[:, hp * 128:(hp + 1) * 128],
                        rhs=dctT_sb[:, c, par * 128:(par + 1) * 128],
                        start=(c == 0),
                        stop=(c == NCH - 1),
                    )
                if after_chunk:
                    after_chunk(c)
            return groups

        def evict_groups(groups, dst_sb, par, engs):
            for (hp, g), e in zip(sorted(groups.items()), engs):
                dst = dst_sb[:, hp, par * 128:(par + 1) * 128]
                if e == "v":
                    nc.vector.tensor_copy(dst, g[:])
                else:
                    nc.scalar.activation(dst, g[:], COPY)

        def evict_groups_bdq(groups, par, engs):
            # group (hp, par) [128 d-pair, 128 m] -> the two diagonal slots
            for (hp, g), e in zip(sorted(groups.items()), engs):
                for i in range(2):
                    dst = bdq[hp][i * 64:(i + 1) * 64, i, par * 128:(par + 1) * 128]
                    src = g[i * 64:(i + 1) * 64, :]
                    if e == "v":
                        nc.vector.tensor_copy(dst, src)
                    else:
                        nc.scalar.activation(dst, src, COPY)

        LO, HI = (0, 1, 2), (3, 4, 5)
        for par in range(MB):
            for hps in (LO, HI):
                g = proj_pass(q_t, par, hps, f"q{par}")
                evict_groups_bdq(g, par, "vsv" if hps is LO else "svs")

        # ---- energy helper: one head-PAIR x one k-block (1 mm + 1 exp) ----
        # E^T(pair)[kb-block, (head, m)] = kdT(kb-block)^T @ bdq: the zero
        # off-diagonal blocks keep the heads separate.
        p_tiles = [None] * HP   # per pair: [128, MB(kb), 2(head), M]

        def emit_pair(hp, kb):
            if p_tiles[hp] is None:
                p_tiles[hp] = pbuf.tile(
                    [128, MB, 2, M], BF, tag=f"p{hp}", name=f"p{hp}"
                )
            pe = psE.tile([128, 2, M], F32, tag="E", name=f"e{hp}{kb}")
            nc.tensor.matmul(
                pe[:],
                lhsT=kdT_sb[:, hp, kb * 128:(kb + 1) * 128],
                rhs=bdq[hp][:],
                start=True,
                stop=True,
            )
            nc.scalar.activation(
                p_tiles[hp][:, kb, :, :], pe[:], EXP, bias=ebias[:]
            )

        # interleave helper: emit energy pairs at odd chunks of a pass
        def pair_seq(pairs, kb):
            def after_chunk(c):
                if c % 2 == 1 and pairs:
                    emit_pair(pairs.pop(0), kb)
            return after_chunk

        # ---- K projections, energies paced into the pass bubbles ----------
        g = proj_pass(k_t, 0, LO, "k0")
        evict_groups(g, kdT_sb, 0, "vvs")   # gates the first energies
        g = proj_pass(k_t, 0, HI, "k0h", pair_seq([0, 1, 2], 0))
        evict_groups(g, kdT_sb, 0, "vvv")   # ACT is running the exp chain
        g = proj_pass(k_t, 1, LO, "k1", pair_seq([3, 4, 5], 0))
        evict_groups(g, kdT_sb, 1, "vvv")
        g = proj_pass(k_t, 1, HI, "k1h", pair_seq([0, 1, 2], 1))
        evict_groups(g, kdT_sb, 1, "vvv")

        # ---- V-proj: one fs column-half at a time --------------------------
        # fs0 sweep (heads 0-5) interleaves the last kb=1 energies; its Vd
        # rows land while the fs1 half of v is still streaming.
        nhp = FW // D  # 6 heads per split
        e1 = [3, 4, 5]

        def v_chunks(fs, vg, cs):
            for c in cs:               # c<8: A chunks; c>=8: B chunks
                par = c // NCH
                nc.tensor.matmul(
                    vg[par][:],
                    lhsT=dctT_sb[:, c % NCH, par * 128:(par + 1) * 128],
                    rhs=v_t[fs][c // QT][:, c % QT, :],
                    start=(c % NCH == 0),
                    stop=(c % NCH == NCH - 1),
                )
                if fs == 0 and c % 2 == 1 and e1:
                    emit_pair(e1.pop(0), 1)
                if c % NCH == NCH - 1:  # evict this parity's Vd rows
                    src = vg[par][:].rearrange("p (h x) -> p h x", x=D)
                    dst = vd_sb[:, par, fs * nhp:(fs + 1) * nhp, 0:D]
                    # last fs1 evict on ACT (idle after the exp chain) so
                    # Vector is free for the phase-B recip chain
                    if fs == 1 and par == 1:
                        nc.scalar.activation(dst, src, COPY)
                    else:
                        nc.vector.tensor_copy(dst, src)

        vg0 = [
            psA.tile([128, FW], F32, tag="A", name=f"vg0{par}")
            for par in range(MB)
        ]
        v_chunks(0, vg0, range(2 * NCH))

        # ---- phase B: ctx = P @ [Vd | 1] then normalize by the sums col ---
        def phase_b(h):
            hp, hi = h // 2, h % 2
            p_t = p_tiles[hp]
            # alternate pools: psE is idle after the exp chain, psA has
            # spare slots during phase B -> two independent rotations keep
            # the per-head normalize chains from serializing
            pool, tg = (psE, "E") if h % 2 == 0 else (psA, "A")
            pc = pool.tile([128, MB, VW], F32, tag=tg, name=f"c{h}")
            for mb in range(MB):
                for kb in range(MB):
                    nc.tensor.matmul(
                        pc[:, mb, :],
                        lhsT=p_t[:, kb, hi, mb * 128:(mb + 1) * 128],
                        rhs=vd_sb[:, kb, h, :],
                        start=(kb == 0),
                        stop=(kb == MB - 1),
                    )
            rs = rbuf.tile([128, MB], F32, tag="r", name=f"r{h}")
            nc.vector.reciprocal(rs[:], pc[:, :, D])
            for mb in range(MB):
                dst = ctx_sb[:, mb, h * D:(h + 1) * D]
                if (h + mb) % 2 == 0:
                    nc.vector.tensor_scalar_mul(
                        dst, pc[:, mb, 0:D], rs[:, mb:mb + 1]
                    )
                else:
                    nc.scalar.activation(
                        dst, pc[:, mb, 0:D], COPY, scale=rs[:, mb:mb + 1]
                    )

        # ---- inverse DCT --------------------------------------------------
        # half 0: x rows nb*128..      accumulate  dct_even@ctx_e + dct_odd@ctx_o
        # half 1: y rows NH+nb*128..   accumulate  dct_even@ctx_e + dctn @ctx_o
        # (host un-reverses the upper rows; the PE does the +- for free in
        # PSUM, copies evict alternating Vector/Scalar)
        def stage3_half(fs, extra=(), skip=0):
            extra = list(extra)
            NB = NH // 128  # 8 row-blocks per half
            # out-DMA pieces; the final piece is small so its transfer
            # starts (and finishes) sooner after the last eviction
            pieces = [(0, 4), (4, 4)] if fs == 0 else [(0, 4), (4, 2), (6, 2)]
            fslice = slice(fs * FW, (fs + 1) * FW)
            slot = 0
            for half in range(2):
                for p0, plen in pieces:
                    ost = ostage.tile(
                        [128, plen, FW], BF,
                        tag=f"o{fs}{half}{p0}", name=f"o{fs}{half}{p0}",
                    )
                    for nbi in range(plen):
                        nb = p0 + nbi
                        px = psA.tile(
                            [128, FW], F32, tag="A", name=f"x{fs}{half}{nb}"
                        )
                        nc.tensor.matmul(
                            px[:],
                            lhsT=dct_sb[:, 0, nb * 128:(nb + 1) * 128],
                            rhs=ctx_sb[:, 0, fslice],
                            start=True,
                            stop=False,
                        )
                        odd_lhsT = (
                            dct_sb[:, 1, nb * 128:(nb + 1) * 128] if half == 0
                            else dctn_sb[:, nb * 128:(nb + 1) * 128]
                        )
                        nc.tensor.matmul(
                            px[:],
                            lhsT=odd_lhsT,
                            rhs=ctx_sb[:, 1, fslice],
                            start=False,
                            stop=True,
                        )
                        # DVE copies are faster than ACT: give DVE 5 of 8
                        if nb % 8 in (1, 4, 6):
                            nc.scalar.activation(ost[:, nbi, :], px[:], COPY)
                        else:
                            nc.vector.tensor_copy(ost[:, nbi, :], px[:])
                        slot += 1
                        if extra and slot > skip:
                            extra.pop(0)()
                    nc.sync.dma_start(
                        out_r[
                            fs,
                            :,
                            half * NB + p0:half * NB + p0 + plen,
                            :,
                        ],
                        ost[:],
                    )

        # phase B h0-5 interleaved with the fs1 V sweep: the fs1 v quarters
        # arrive exactly while the recip/normalize chains pace phase B, so
        # the fs1 projection rides in the PE bubbles.
        vg1 = [
            psA.tile([128, FW], F32, tag="A", name=f"vg1{par}")
            for par in range(MB)
        ]
        for h in range(6):   # heads 0..5 feed fs0
            phase_b(h)
            lo = h * 3 if h < 5 else 15
            v_chunks(1, vg1, range(lo, min(lo + 3, 16)) if h < 5 else [15])
        # interleave heads 6..11 into the fs0 sweep so the PE never waits on
        # the normalize chain
        extra = [(lambda hh=h: phase_b(hh)) for h in range(6, H)]
        stage3_half(0, extra, skip=2)
        stage3_half(1)

    nc.compile()
    return nc


def prep_in_maps(Q, K, V, mask, Q_dct):
    Q, K, V = np.asarray(Q), np.asarray(K), np.asarray(V)
    mask, Q_dct = np.asarray(mask), np.asarray(Q_dct)
    scale = np.float32(1.0 / np.sqrt(np.sqrt(np.float32(D))))
    m4 = mask.astype(np.float32)[:, None, :, None]        # [B,1,N,1]

    def fold(x):  # [B,N,HD] -> [A; B] along N
        lo, hi = x[:, :NH, :], x[:, NH:, :][:, ::-1, :]
        return np.concatenate([lo + hi, lo - hi], axis=1)

    def ptile(x, nch):  # [B, nch*128, F] -> [B, 128, nch, F] flat: row p*nch+c
        b, n, f = x.shape
        return x.reshape(b, nch, 128, f).transpose(0, 2, 1, 3).reshape(b, n, f)

    qs = fold((Q.astype(np.float32) * scale).transpose(0, 2, 1, 3).reshape(B, N, HD))
    ks = fold((K.astype(np.float32) * scale * m4).transpose(0, 2, 1, 3).reshape(B, N, HD))
    vs = fold((V.astype(np.float32) * m4).transpose(0, 2, 1, 3).reshape(B, N, HD))
    qs = np.ascontiguousarray(ptile(qs, 2 * NCH)).astype(_BF16)
    ks = np.ascontiguousarray(ptile(ks, 2 * NCH)).astype(_BF16)
    # v fs-major: [B, fs, p, c, FW] flattened to [B, 2N, FW]
    vs = np.stack(
        [ptile(vs[:, :, f * FW:(f + 1) * FW], 2 * NCH) for f in range(FSPLIT)],
        axis=1,
    ).reshape(B, 2 * N, FW)
    vs = np.ascontiguousarray(vs).astype(_BF16)

    dct_f = Q_dct.astype(np.float32)
    perm = np.concatenate([np.arange(0, M, 2), np.arange(1, M, 2)])
    dct_p = dct_f[perm]                            # rows reordered [even|odd]
    dctT = dct_p[:, :NH].T                         # [NH, M]
    dctT = np.ascontiguousarray(ptile(dctT[None], NCH)[0]).astype(_BF16)
    return [
        {"q": qs[b], "k": ks[b], "v": vs[b], "dctT": dctT}
        for b in range(B)
    ]


def unpack_out(out_arr):
    """Device 'out' [2*N, FW] (layout [fs, p, c-block, w]) -> [H, N, D]."""
    o = np.asarray(out_arr).astype(np.float32)
    o = o.reshape(FSPLIT, 128, 2 * NCH, FW)
    o = o.transpose(2, 1, 0, 3).reshape(N, HD)   # row c*128+p
    o[NH:] = o[NH:][::-1]   # upper rows hold y[j] = x[N-1-j]
    return o.reshape(N, H, D).transpose(1, 0, 2)


def run(Q, K, V, mask, Q_dct, trace=False):
    from concourse.bass_utils import run_bass_kernel_spmd

    if "nc" not in _CACHE:
        _CACHE["nc"] = build_nc()
    nc = _CACHE["nc"]
    in_maps = prep_in_maps(Q, K, V, mask, Q_dct)
    res = run_bass_kernel_spmd(nc, in_maps, core_ids=list(range(B)), trace=trace)
    x = np.stack([unpack_out(res.results[i]["out"]) for i in range(B)])
    return np.ascontiguousarray(x, dtype=np.float32), res


def kernel(Q, K, V, mask, Q_dct):
    x, _ = run(Q, K, V, mask, Q_dct, trace=False)
    return x


# revision 57
# speedup vs baseline: 1.1361x; 1.1361x over previous
"""DCT-attention Trainium2 kernel (8 NeuronCores, data-parallel over batch).

Reference math (per b, h):
    Qd = dct @ (Q*s);  Kd = dct @ (K*s*mask);  Vd = dct @ (V*mask)   # [M,D]
    E  = Qd @ Kd^T;  P = softmax(E, axis=-1);  ctx = P @ Vd          # [M,D]
    x  = dct^T @ ctx                                                 # [N,D]
with B,H,N,D = 8,12,2048,64, M = 256, s = D**-0.25.

Sharding: batch b -> core b (8 cores). Host folds scale into Q/K and mask into
K/V, transposes to [N, H*D], bf16-casts; matmuls run bf16 -> fp32 PSUM; output
returns bf16 and is cast to f32 on the host.

DCT parity symmetry: dct[k, N-1-i] = (-1)^k dct[k, i].  The host uploads X
folded as [A; B] with A = X[:N/2] + reverse(X[N/2:]), B = X[:N/2] -
reverse(X[N/2:]), and the M axis globally reordered to [even k | odd k].
Every projection then contracts over N/2 instead of N.

Differences vs the first working version (67.0us):
  * dct (inverse basis, 0.5MB) is no longer uploaded; it is built on-chip by
    16 PE transposes of dctT blocks during the PE bubble while k streams in.
    A negated copy of the odd-k rows (dctn) is evicted in the same window so
    the upper output half accumulates E-O without any back-half negate ops.
  * energies are emitted per HEAD-PAIR into one PSUM bank ([128,2,256]) and
    exp'd in a single ACT op per pair -> the serial exp chain halves
    (24 x ~480ns -> 12 x ~650ns).
  * phase B uses one PSUM tile per head for both m-blocks -> one reciprocal
    per head instead of two.
  * out-DMA issue alternates Sync (HWDGE) and GpSimd (SWDGE) queues; GpSimd
    cannot touch PSUM on TRN2, so descriptor generation is the only useful
    back-half work for it.
  * evictions/normalize split across Vector/Scalar so neither chain gates
    the PE.
"""

import numpy as np
import ml_dtypes

B, H, N, D = 8, 12, 2048, 64
M = 256
HD = H * D          # 768
NH = N // 2         # 1024 folded length
NCH = NH // 128     # 8 folded chunks per parity phase
QT = 4              # chunks per input-DMA quarter (of the folded [N, HD] array)
MB = M // 128       # 2 m-blocks (even ks | odd ks)
HP = H // 2         # 6 head-pairs
FSPLIT = 2          # HD split for <=512-wide psum
FW = HD // FSPLIT   # 384
VW = D + 1          # 65: Vd columns + ones column

_BF16 = ml_dtypes.bfloat16
_CACHE = {}


def build_nc():
    import concourse.bacc as bacc
    import concourse.mybir as mybir
    import concourse.tile as tile
    from concourse.masks import make_identity
    from contextlib import ExitStack

    BF = mybir.dt.bfloat16
    F32 = mybir.dt.float32
    EXP = mybir.ActivationFunctionType.Exp
    COPY = mybir.ActivationFunctionType.Copy

    nc = bacc.Bacc()
    q_d = nc.declare_dram_parameter("q", [N, HD], BF, isOutput=False)
    k_d = nc.declare_dram_parameter("k", [N, HD], BF, isOutput=False)
    # v is uploaded fs-major ([fs, p, c, FW]) so the head-0-5 column half
    # arrives first: phase B h0-5 and the fs0 inverse DCT then overlap the
    # tail of the v stream.
    v_d = nc.declare_dram_parameter("v", [2 * N, FW], BF, isOutput=False)
    # dctT: [n' < N/2, m] with columns [even k | odd k]
    dctT_d = nc.declare_dram_parameter("dctT", [NH, M], BF, isOutput=False)
    out_d = nc.declare_dram_parameter("out", [2 * N, FW], BF, isOutput=True)

    # DRAM layouts are pre-tiled on the host to [partition, chunk, free]
    # so every DMA descriptor covers a 6KB (q/k/v) / 4KB (dctT) / 3KB (out)
    # contiguous run instead of 1.5KB rows scattered by the partition
    # interleave -- fewer descriptors, better sustained HBM rate.
    q_r = q_d.ap().rearrange("(p c) f -> p c f", c=2 * NCH)
    k_r = k_d.ap().rearrange("(p c) f -> p c f", c=2 * NCH)
    v_r = v_d.ap().rearrange("(f p c) w -> f p c w", f=FSPLIT, c=2 * NCH)
    dctT_r = dctT_d.ap().rearrange("(p c) m -> p c m", c=NCH)
    # out: [fs, p, c, FW] flattened to [N, HD] -- fs-major halves
    out_r = out_d.ap().rearrange("(f p c) w -> f p c w", f=FSPLIT, c=2 * NCH)

    with ExitStack() as ctx:
        tc = ctx.enter_context(tile.TileContext(nc))
        consts = ctx.enter_context(tc.tile_pool(name="consts", bufs=1))
        xin = ctx.enter_context(tc.tile_pool(name="xin", bufs=1))
        proj = ctx.enter_context(tc.tile_pool(name="proj", bufs=1))
        pbuf = ctx.enter_context(tc.tile_pool(name="pbuf", bufs=1))
        rbuf = ctx.enter_context(tc.tile_pool(name="rbuf", bufs=8))
        ostage = ctx.enter_context(tc.tile_pool(name="ostage", bufs=1))
        psA = ctx.enter_context(tc.tile_pool(name="psA", bufs=5, space="PSUM"))
        psE = ctx.enter_context(tc.tile_pool(name="psE", bufs=3, space="PSUM"))

        # ---- on-chip constants (no DMA) -----------------------------------
        ident = consts.tile([128, 128], BF)
        make_identity(nc, ident)
        ebias = consts.tile([128, 1], F32)
        nc.gpsimd.memset(ebias[:], -4.0)

        # ---- DMA stream (single FIFO queue; order == consumption order) ----
        # dctT split in two so Q-proj chunk 0 can start half a transfer early
        dctT_sb = consts.tile([128, NCH, M], BF)       # [n'-part, chunk, m]
        nc.sync.dma_start(dctT_sb[:, 0:NCH // 2, :], dctT_r[:, 0:NCH // 2, :])
        nc.sync.dma_start(dctT_sb[:, NCH // 2:, :], dctT_r[:, NCH // 2:, :])

        def quarters(name, src_r):
            tiles = []
            for qt in range(2 * NCH // QT):
                t = xin.tile([128, QT, HD], BF, tag=f"{name}{qt}")
                cs = slice(qt * QT, (qt + 1) * QT)
                nc.sync.dma_start(t[:], src_r[:, cs, :])
                tiles.append(t)
            return tiles

        q_t = quarters("q", q_r)   # chunks 0..7 = A-fold, 8..15 = B-fold
        k_t = quarters("k", k_r)
        # v: fs-major quarters ([128, QT, FW] each), fs0 then fs1
        v_t = [[], []]
        for fs in range(FSPLIT):
            for qt in range(2 * NCH // QT):
                t = xin.tile([128, QT, FW], BF, tag=f"v{fs}{qt}")
                cs = slice(qt * QT, (qt + 1) * QT)
                nc.sync.dma_start(t[:], v_r[fs, :, cs, :])
                v_t[fs].append(t)

        # ---- persistent intermediates ----
        # bdq: block-diagonal qdT per head-pair: [[qdT_h0, 0], [0, qdT_h1]]
        # ([128 d-pair rows, (head, m) cols]).  One full-128-partition matmul
        # against kdT then yields BOTH heads' energies side by side in one
        # PSUM bank, and one 512-wide exp covers the pair -> the serial ACT
        # exp chain halves (24 ops -> 12).
        bdq = [
            proj.tile([128, 2, M], BF, tag=f"bdq{hp}", name=f"bdq{hp}")
            for hp in range(HP)
        ]
        for hp in range(HP):
            nc.gpsimd.memset(bdq[hp][:], 0.0)
        kdT_sb = proj.tile([128, HP, M], BF, tag="kdT")
        vd_sb = proj.tile([128, MB, H, VW], BF, tag="vd")  # [m-part, mb, h, d+1]
        ctx_sb = proj.tile([128, MB, HD], BF, tag="ctx")   # [m-part, mb, h*d]
        # dct: [m-part, m-block, n'] built on-chip from dctT via PE transpose
        dct_sb = consts.tile([128, MB, NH], BF)
        nc.vector.memset(vd_sb[:, :, :, D:VW], 1.0)

        def xc(tiles, c):  # folded chunk c (0..15), [128, HD]
            return tiles[c // QT][:, c % QT, :]

        # eviction engine rotation (psum -> sbuf copies; GpSimd cannot
        # read PSUM on TRN2, so only Vector and Scalar qualify)
        def make_rot(*engines):
            state = {"i": 0}

            def rot(dst, src):
                e = engines[state["i"] % len(engines)]
                state["i"] += 1
                if e == "v":
                    nc.vector.tensor_copy(dst, src)
                else:
                    nc.scalar.activation(dst, src, COPY)
            return rot

        # ---- build dct (inverse basis) by transposing dctT blocks ---------
        # first thing on the PE: only needs dctT, runs while q streams in.
        # odd-k rows (mbb=1) are additionally evicted negated into dctn_sb
        # for the E-O accumulation of the reconstructed upper output half.
        dctn_sb = consts.tile([128, NH], BF)
        for c in range(NCH):
            for mbb in range(MB):
                pt = psA.tile([128, 128], BF, tag="A", name=f"t{c}{mbb}")
                nc.tensor.transpose(
                    pt[:], dctT_sb[:, c, mbb * 128:(mbb + 1) * 128], ident[:]
                )
                # balance the evict chain across both engines so the 5-deep
                # psA rotation never stalls the transpose stream
                if (c + mbb) % 2 == 0:
                    nc.vector.tensor_copy(
                        dct_sb[:, mbb, c * 128:(c + 1) * 128], pt[:]
                    )
                else:
                    nc.scalar.activation(
                        dct_sb[:, mbb, c * 128:(c + 1) * 128], pt[:], COPY
                    )
                if mbb == 1:
                    if c % 2 == 0:
                        nc.vector.tensor_scalar_mul(
                            dctn_sb[:, c * 128:(c + 1) * 128], pt[:], -1.0
                        )
                    else:
                        nc.scalar.activation(
                            dctn_sb[:, c * 128:(c + 1) * 128], pt[:], COPY,
                            scale=-1.0,
                        )

        # ---- Q/K projections ----------------------------------------------
        # Each parity phase runs as TWO chunk-major passes over 3 head-pair
        # groups (instead of one pass over 6):  the second pass refills the
        # chase bubbles left by the DMA pacing, frees a PSUM bank (psA=5,
        # psE=3 -> the exp chain never waits on the psE ping-pong), and for
        # K lets the first head-pairs evict one pass earlier, which starts
        # the serial exp chain sooner.
        def proj_pass(tiles, par, hps, tagn, after_chunk=None):
            groups = {
                hp: psA.tile([128, 128], F32, tag="A", name=f"g{tagn}{hp}")
                for hp in hps
            }
            for c in range(NCH):
                for hp in hps:
                    nc.tensor.matmul(
                        groups[hp][:],
                        lhsT=xc(tiles, par * NCH + c)[:, hp * 128:(hp + 1) * 128],
                        rhs=dctT_sb[:, c, par * 128:(par + 1) * 128],
                        start=(c == 0),
                        stop=(c == NCH - 1),
                    )
                if after_chunk:
                    after_chunk(c)
            return groups

        def evict_groups(groups, dst_sb, par, engs):
            for (hp, g), e in zip(sorted(groups.items()), engs):
                dst = dst_sb[:, hp, par * 128:(par + 1) * 128]
                if e == "v":
                    nc.vector.tensor_copy(dst, g[:])
                else:
                    nc.scalar.activation(dst, g[:], COPY)

        def evict_groups_bdq(groups, par, engs):
            # group (hp, par) [128 d-pair, 128 m] -> the two diagonal slots
            for (hp, g), e in zip(sorted(groups.items()), engs):
                for i in range(2):
                    dst = bdq[hp][i * 64:(i + 1) * 64, i, par * 128:(par + 1) * 128]
                    src = g[i * 64:(i + 1) * 64, :]
                    if e == "v":
                        nc.vector.tensor_copy(dst, src)
                    else:
                        nc.scalar.activation(dst, src, COPY)

        LO, HI = (0, 1, 2), (3, 4, 5)
        for par in range(MB):
            for hps in (LO, HI):
                g = proj_pass(q_t, par, hps, f"q{par}")
                evict_groups_bdq(g, par, "vsv" if hps is LO else "svs")

        # ---- energy helper: one head-PAIR x one k-block (1 mm + 1 exp) ----
        # E^T(pair)[kb-block, (head, m)] = kdT(kb-block)^T @ bdq: the zero
        # off-diagonal blocks keep the heads separate.
        p_tiles = [None] * HP   # per pair: [128, MB(kb), 2(head), M]

        def emit_pair(hp, kb):
            if p_tiles[hp] is None:
                p_tiles[hp] = pbuf.tile(
                    [128, MB, 2, M], BF, tag=f"p{hp}", name=f"p{hp}"
                )
            pe = psE.tile([128, 2, M], F32, tag="E", name=f"e{hp}{kb}")
            nc.tensor.matmul(
                pe[:],
                lhsT=kdT_sb[:, hp, kb * 128:(kb + 1) * 128],
                rhs=bdq[hp][:],
                start=True,
                stop=True,
            )
            nc.scalar.activation(
                p_tiles[hp][:, kb, :, :], pe[:], EXP, bias=ebias[:]
            )

        # interleave helper: emit energy pairs at odd chunks of a pass
        def pair_seq(pairs, kb):
            def after_chunk(c):
                if c % 2 == 1 and pairs:
                    emit_pair(pairs.pop(0), kb)
            return after_chunk

        # ---- K projections, energies paced into the pass bubbles ----------
        g = proj_pass(k_t, 0, LO, "k0")
        evict_groups(g, kdT_sb, 0, "vvs")   # gates the first energies
        g = proj_pass(k_t, 0, HI, "k0h", pair_seq([0, 1, 2], 0))
        evict_groups(g, kdT_sb, 0, "vvv")   # ACT is running the exp chain
        g = proj_pass(k_t, 1, LO, "k1", pair_seq([3, 4, 5], 0))
        evict_groups(g, kdT_sb, 1, "vvv")
        g = proj_pass(k_t, 1, HI, "k1h", pair_seq([0, 1, 2], 1))
        evict_groups(g, kdT_sb, 1, "vvv")

        # ---- V-proj: one fs column-half at a time --------------------------
        # fs0 sweep (heads 0-5) interleaves the last kb=1 energies; its Vd
        # rows land while the fs1 half of v is still streaming.
        nhp = FW // D  # 6 heads per split
        e1 = [3, 4, 5]

        def v_chunks(fs, vg, cs):
            for c in cs:               # c<8: A chunks; c>=8: B chunks
                par = c // NCH
                nc.tensor.matmul(
                    vg[par][:],
                    lhsT=dctT_sb[:, c % NCH, par * 128:(par + 1) * 128],
                    rhs=v_t[fs][c // QT][:, c % QT, :],
                    start=(c % NCH == 0),
                    stop=(c % NCH == NCH - 1),
                )
                if fs == 0 and c % 2 == 1 and e1:
                    emit_pair(e1.pop(0), 1)
                if c % NCH == NCH - 1:  # evict this parity's Vd rows
                    src = vg[par][:].rearrange("p (h x) -> p h x", x=D)
                    dst = vd_sb[:, par, fs * nhp:(fs + 1) * nhp, 0:D]
                    # last fs1 evict on ACT (idle after the exp chain) so
                    # Vector is free for the phase-B recip chain
                    if fs == 1 and par == 1:
                        nc.scalar.activation(dst, src, COPY)
                    else:
                        nc.vector.tensor_copy(dst, src)

        vg0 = [
            psA.tile([128, FW], F32, tag="A", name=f"vg0{par}")
            for par in range(MB)
        ]
        v_chunks(0, vg0, range(2 * NCH))

        # ---- phase B: ctx = P @ [Vd | 1] then normalize by the sums col ---
        def phase_b(h):
            hp, hi = h // 2, h % 2
            p_t = p_tiles[hp]
            # psE pool (idle after the exp chain): frees psA for the fs1 V
            # groups and the inverse-DCT rotation
            pc = psE.tile([128, MB, VW], F32, tag="E", name=f"c{h}")
            for mb in range(MB):
                for kb in range(MB):
                    nc.tensor.matmul(
                        pc[:, mb, :],
                        lhsT=p_t[:, kb, hi, mb * 128:(mb + 1) * 128],
                        rhs=vd_sb[:, kb, h, :],
                        start=(kb == 0),
                        stop=(kb == MB - 1),
                    )
            rs = rbuf.tile([128, MB], F32, tag="r", name=f"r{h}")
            nc.vector.reciprocal(rs[:], pc[:, :, D])
            for mb in range(MB):
                dst = ctx_sb[:, mb, h * D:(h + 1) * D]
                if (h + mb) % 2 == 0:
                    nc.vector.tensor_scalar_mul(
                        dst, pc[:, mb, 0:D], rs[:, mb:mb + 1]
                    )
                else:
                    nc.scalar.activation(
                        dst, pc[:, mb, 0:D], COPY, scale=rs[:, mb:mb + 1]
                    )

        # ---- inverse DCT --------------------------------------------------
        # half 0: x rows nb*128..      accumulate  dct_even@ctx_e + dct_odd@ctx_o
        # half 1: y rows NH+nb*128..   accumulate  dct_even@ctx_e + dctn @ctx_o
        # (host un-reverses the upper rows; the PE does the +- for free in
        # PSUM, copies evict alternating Vector/Scalar)
        def stage3_half(fs, extra=(), skip=0):
            extra = list(extra)
            NB = NH // 128  # 8 row-blocks per half
            # out-DMA pieces; the final piece is small so its transfer
            # starts (and finishes) sooner after the last eviction
            pieces = [(0, 4), (4, 4)] if fs == 0 else [(0, 4), (4, 2), (6, 2)]
            fslice = slice(fs * FW, (fs + 1) * FW)
            slot = 0
            for half in range(2):
                for p0, plen in pieces:
                    ost = ostage.tile(
                        [128, plen, FW], BF,
                        tag=f"o{fs}{half}{p0}", name=f"o{fs}{half}{p0}",
                    )
                    for nbi in range(plen):
                        nb = p0 + nbi
                        px = psA.tile(
                            [128, FW], F32, tag="A", name=f"x{fs}{half}{nb}"
                        )
                        nc.tensor.matmul(
                            px[:],
                            lhsT=dct_sb[:, 0, nb * 128:(nb + 1) * 128],
                            rhs=ctx_sb[:, 0, fslice],
                            start=True,
                            stop=False,
                        )
                        odd_lhsT = (
                            dct_sb[:, 1, nb * 128:(nb + 1) * 128] if half == 0
                            else dctn_sb[:, nb * 128:(nb + 1) * 128]
                        )
                        nc.tensor.matmul(
                            px[:],
                            lhsT=odd_lhsT,
                            rhs=ctx_sb[:, 1, fslice],
                            start=False,
                            stop=True,
                        )
                        # DVE copies are faster than ACT: give DVE 5 of 8
                        if nb % 8 in (1, 4, 6):
                            nc.scalar.activation(ost[:, nbi, :], px[:], COPY)
                        else:
                            nc.vector.tensor_copy(ost[:, nbi, :], px[:])
                        slot += 1
                        if extra and slot > skip:
                            extra.pop(0)()
                    nc.sync.dma_start(
                        out_r[
                            fs,
                            :,
                            half * NB + p0:half * NB + p0 + plen,
                            :,
                        ],
                        ost[:],
                    )

        # phase B h0-5 interleaved with the fs1 V sweep: the fs1 v quarters
        # arrive exactly while the recip/normalize chains pace phase B, so
        # the fs1 projection rides in the PE bubbles.
        vg1 = [
            psA.tile([128, FW], F32, tag="A", name=f"vg1{par}")
            for par in range(MB)
        ]
        for h in range(6):   # heads 0..5 feed fs0
            phase_b(h)
            lo = h * 3 if h < 5 else 15
            v_chunks(1, vg1, range(lo, min(lo + 3, 16)) if h < 5 else [15])
        # interleave heads 6..11 into the fs0 sweep so the PE never waits on
        # the normalize chain
        extra = [(lambda hh=h: phase_b(hh)) for h in range(6, H)]
        stage3_half(0, extra, skip=2)
        stage3_half(1)

    nc.compile()
    return nc


def prep_in_maps(Q, K, V, mask, Q_dct):
    Q, K, V = np.asarray(Q), np.asarray(K), np.asarray(V)
    mask, Q_dct = np.asarray(mask), np.asarray(Q_dct)
    scale = np.float32(1.0 / np.sqrt(np.sqrt(np.float32(D))))
    m4 = mask.astype(np.float32)[:, None, :, None]        # [B,1,N,1]

    def fold(x):  # [B,N,HD] -> [A; B] along N
        lo, hi = x[:, :NH, :], x[:, NH:, :][:, ::-1, :]
        return np.concatenate([lo + hi, lo - hi], axis=1)

    def ptile(x, nch):  # [B, nch*128, F] -> [B, 128, nch, F] flat: row p*nch+c
        b, n, f = x.shape
        return x.reshape(b, nch, 128, f).transpose(0, 2, 1, 3).reshape(b, n, f)

    qs = fold((Q.astype(np.float32) * scale).transpose(0, 2, 1, 3).reshape(B, N, HD))
    ks = fold((K.astype(np.float32) * scale * m4).transpose(0, 2, 1, 3).reshape(B, N, HD))
    vs = fold((V.astype(np.float32) * m4).transpose(0, 2, 1, 3).reshape(B, N, HD))
    qs = np.ascontiguousarray(ptile(qs, 2 * NCH)).astype(_BF16)
    ks = np.ascontiguousarray(ptile(ks, 2 * NCH)).astype(_BF16)
    # v fs-major: [B, fs, p, c, FW] flattened to [B, 2N, FW]
    vs = np.stack(
        [ptile(vs[:, :, f * FW:(f + 1) * FW], 2 * NCH) for f in range(FSPLIT)],
        axis=1,
    ).reshape(B, 2 * N, FW)
    vs = np.ascontiguousarray(vs).astype(_BF16)

    dct_f = Q_dct.astype(np.float32)
    perm = np.concatenate([np.arange(0, M, 2), np.arange(1, M, 2)])
    dct_p = dct_f[perm]                            # rows reordered [even|odd]
    dctT = dct_p[:, :NH].T                         # [NH, M]
    dctT = np.ascontiguousarray(ptile(dctT[None], NCH)[0]).astype(_BF16)
    return [
        {"q": qs[b], "k": ks[b], "v": vs[b], "dctT": dctT}
        for b in range(B)
    ]


def unpack_out(out_arr):
    """Device 'out' [2*N, FW] (layout [fs, p, c-block, w]) -> [H, N, D]."""
    o = np.asarray(out_arr).astype(np.float32)
    o = o.reshape(FSPLIT, 128, 2 * NCH, FW)
    o = o.transpose(2, 1, 0, 3).reshape(N, HD)   # row c*128+p
    o[NH:] = o[NH:][::-1]   # upper rows hold y[j] = x[N-1-j]
    return o.reshape(N, H, D).transpose(1, 0, 2)


def run(Q, K, V, mask, Q_dct, trace=False):
    from concourse.bass_utils import run_bass_kernel_spmd

    if "nc" not in _CACHE:
        _CACHE["nc"] = build_nc()
    nc = _CACHE["nc"]
    in_maps = prep_in_maps(Q, K, V, mask, Q_dct)
    res = run_bass_kernel_spmd(nc, in_maps, core_ids=list(range(B)), trace=trace)
    x = np.stack([unpack_out(res.results[i]["out"]) for i in range(B)])
    return np.ascontiguousarray(x, dtype=np.float32), res


def kernel(Q, K, V, mask, Q_dct):
    x, _ = run(Q, K, V, mask, Q_dct, trace=False)
    return x


# revision 58
# speedup vs baseline: 1.1392x; 1.0028x over previous
"""DCT-attention Trainium2 kernel (8 NeuronCores, data-parallel over batch).

Reference math (per b, h):
    Qd = dct @ (Q*s);  Kd = dct @ (K*s*mask);  Vd = dct @ (V*mask)   # [M,D]
    E  = Qd @ Kd^T;  P = softmax(E, axis=-1);  ctx = P @ Vd          # [M,D]
    x  = dct^T @ ctx                                                 # [N,D]
with B,H,N,D = 8,12,2048,64, M = 256, s = D**-0.25.

Sharding: batch b -> core b (8 cores). Host folds scale into Q/K and mask into
K/V, transposes to [N, H*D], bf16-casts; matmuls run bf16 -> fp32 PSUM; output
returns bf16 and is cast to f32 on the host.

DCT parity symmetry: dct[k, N-1-i] = (-1)^k dct[k, i].  The host uploads X
folded as [A; B] with A = X[:N/2] + reverse(X[N/2:]), B = X[:N/2] -
reverse(X[N/2:]), and the M axis globally reordered to [even k | odd k].
Every projection then contracts over N/2 instead of N.

Differences vs the first working version (67.0us):
  * dct (inverse basis, 0.5MB) is no longer uploaded; it is built on-chip by
    16 PE transposes of dctT blocks during the PE bubble while k streams in.
    A negated copy of the odd-k rows (dctn) is evicted in the same window so
    the upper output half accumulates E-O without any back-half negate ops.
  * energies are emitted per HEAD-PAIR into one PSUM bank ([128,2,256]) and
    exp'd in a single ACT op per pair -> the serial exp chain halves
    (24 x ~480ns -> 12 x ~650ns).
  * phase B uses one PSUM tile per head for both m-blocks -> one reciprocal
    per head instead of two.
  * out-DMA issue alternates Sync (HWDGE) and GpSimd (SWDGE) queues; GpSimd
    cannot touch PSUM on TRN2, so descriptor generation is the only useful
    back-half work for it.
  * evictions/normalize split across Vector/Scalar so neither chain gates
    the PE.
"""

import numpy as np
import ml_dtypes

B, H, N, D = 8, 12, 2048, 64
M = 256
HD = H * D          # 768
NH = N // 2         # 1024 folded length
NCH = NH // 128     # 8 folded chunks per parity phase
QT = 4              # chunks per input-DMA quarter (of the folded [N, HD] array)
MB = M // 128       # 2 m-blocks (even ks | odd ks)
HP = H // 2         # 6 head-pairs
FSPLIT = 2          # HD split for <=512-wide psum
FW = HD // FSPLIT   # 384
VW = D + 1          # 65: Vd columns + ones column

_BF16 = ml_dtypes.bfloat16
_CACHE = {}


def build_nc():
    import concourse.bacc as bacc
    import concourse.mybir as mybir
    import concourse.tile as tile
    from concourse.masks import make_identity
    from contextlib import ExitStack

    BF = mybir.dt.bfloat16
    F32 = mybir.dt.float32
    EXP = mybir.ActivationFunctionType.Exp
    COPY = mybir.ActivationFunctionType.Copy

    nc = bacc.Bacc()
    q_d = nc.declare_dram_parameter("q", [N, HD], BF, isOutput=False)
    k_d = nc.declare_dram_parameter("k", [N, HD], BF, isOutput=False)
    # v is uploaded fs-major ([fs, p, c, FW]) so the head-0-5 column half
    # arrives first: phase B h0-5 and the fs0 inverse DCT then overlap the
    # tail of the v stream.
    v_d = nc.declare_dram_parameter("v", [2 * N, FW], BF, isOutput=False)
    # dctT: [n' < N/2, m] with columns [even k | odd k]
    dctT_d = nc.declare_dram_parameter("dctT", [NH, M], BF, isOutput=False)
    out_d = nc.declare_dram_parameter("out", [2 * N, FW], BF, isOutput=True)

    # DRAM layouts are pre-tiled on the host to [partition, chunk, free]
    # so every DMA descriptor covers a 6KB (q/k/v) / 4KB (dctT) / 3KB (out)
    # contiguous run instead of 1.5KB rows scattered by the partition
    # interleave -- fewer descriptors, better sustained HBM rate.
    q_r = q_d.ap().rearrange("(p c) f -> p c f", c=2 * NCH)
    k_r = k_d.ap().rearrange("(p c) f -> p c f", c=2 * NCH)
    v_r = v_d.ap().rearrange("(f p c) w -> f p c w", f=FSPLIT, c=2 * NCH)
    dctT_r = dctT_d.ap().rearrange("(p c) m -> p c m", c=NCH)
    # out: [fs, p, c, FW] flattened to [N, HD] -- fs-major halves
    out_r = out_d.ap().rearrange("(f p c) w -> f p c w", f=FSPLIT, c=2 * NCH)

    with ExitStack() as ctx:
        tc = ctx.enter_context(tile.TileContext(nc))
        consts = ctx.enter_context(tc.tile_pool(name="consts", bufs=1))
        xin = ctx.enter_context(tc.tile_pool(name="xin", bufs=1))
        proj = ctx.enter_context(tc.tile_pool(name="proj", bufs=1))
        pbuf = ctx.enter_context(tc.tile_pool(name="pbuf", bufs=1))
        rbuf = ctx.enter_context(tc.tile_pool(name="rbuf", bufs=8))
        ostage = ctx.enter_context(tc.tile_pool(name="ostage", bufs=1))
        psA = ctx.enter_context(tc.tile_pool(name="psA", bufs=5, space="PSUM"))
        psE = ctx.enter_context(tc.tile_pool(name="psE", bufs=3, space="PSUM"))

        # ---- on-chip constants (no DMA) -----------------------------------
        ident = consts.tile([128, 128], BF)
        make_identity(nc, ident)
        ebias = consts.tile([128, 1], F32)
        nc.gpsimd.memset(ebias[:], -4.0)

        # ---- DMA stream (single FIFO queue; order == consumption order) ----
        # dctT split in two so Q-proj chunk 0 can start half a transfer early
        dctT_sb = consts.tile([128, NCH, M], BF)       # [n'-part, chunk, m]
        nc.sync.dma_start(dctT_sb[:, 0:NCH // 2, :], dctT_r[:, 0:NCH // 2, :])
        nc.sync.dma_start(dctT_sb[:, NCH // 2:, :], dctT_r[:, NCH // 2:, :])

        def quarters(name, src_r):
            tiles = []
            for qt in range(2 * NCH // QT):
                t = xin.tile([128, QT, HD], BF, tag=f"{name}{qt}")
                cs = slice(qt * QT, (qt + 1) * QT)
                nc.sync.dma_start(t[:], src_r[:, cs, :])
                tiles.append(t)
            return tiles

        q_t = quarters("q", q_r)   # chunks 0..7 = A-fold, 8..15 = B-fold
        k_t = quarters("k", k_r)
        # v: fs-major quarters ([128, QT, FW] each), fs0 then fs1
        v_t = [[], []]
        for fs in range(FSPLIT):
            for qt in range(2 * NCH // QT):
                t = xin.tile([128, QT, FW], BF, tag=f"v{fs}{qt}")
                cs = slice(qt * QT, (qt + 1) * QT)
                nc.sync.dma_start(t[:], v_r[fs, :, cs, :])
                v_t[fs].append(t)

        # ---- persistent intermediates ----
        # bdq: block-diagonal qdT per head-pair: [[qdT_h0, 0], [0, qdT_h1]]
        # ([128 d-pair rows, (head, m) cols]).  One full-128-partition matmul
        # against kdT then yields BOTH heads' energies side by side in one
        # PSUM bank, and one 512-wide exp covers the pair -> the serial ACT
        # exp chain halves (24 ops -> 12).
        bdq = [
            proj.tile([128, 2, M], BF, tag=f"bdq{hp}", name=f"bdq{hp}")
            for hp in range(HP)
        ]
        for hp in range(HP):
            nc.gpsimd.memset(bdq[hp][:], 0.0)
        kdT_sb = proj.tile([128, HP, M], BF, tag="kdT")
        vd_sb = proj.tile([128, MB, H, VW], BF, tag="vd")  # [m-part, mb, h, d+1]
        ctx_sb = proj.tile([128, MB, HD], BF, tag="ctx")   # [m-part, mb, h*d]
        # dct: [m-part, m-block, n'] built on-chip from dctT via PE transpose
        dct_sb = consts.tile([128, MB, NH], BF)
        nc.vector.memset(vd_sb[:, :, :, D:VW], 1.0)

        def xc(tiles, c):  # folded chunk c (0..15), [128, HD]
            return tiles[c // QT][:, c % QT, :]

        # eviction engine rotation (psum -> sbuf copies; GpSimd cannot
        # read PSUM on TRN2, so only Vector and Scalar qualify)
        def make_rot(*engines):
            state = {"i": 0}

            def rot(dst, src):
                e = engines[state["i"] % len(engines)]
                state["i"] += 1
                if e == "v":
                    nc.vector.tensor_copy(dst, src)
                else:
                    nc.scalar.activation(dst, src, COPY)
            return rot

        # ---- build dct (inverse basis) by transposing dctT blocks ---------
        # first thing on the PE: only needs dctT, runs while q streams in.
        # odd-k rows (mbb=1) are additionally evicted negated into dctn_sb
        # for the E-O accumulation of the reconstructed upper output half.
        dctn_sb = consts.tile([128, NH], BF)
        for c in range(NCH):
            for mbb in range(MB):
                pt = psA.tile([128, 128], BF, tag="A", name=f"t{c}{mbb}")
                nc.tensor.transpose(
                    pt[:], dctT_sb[:, c, mbb * 128:(mbb + 1) * 128], ident[:]
                )
                # balance the evict chain across both engines so the 5-deep
                # psA rotation never stalls the transpose stream
                if (c + mbb) % 2 == 0:
                    nc.vector.tensor_copy(
                        dct_sb[:, mbb, c * 128:(c + 1) * 128], pt[:]
                    )
                else:
                    nc.scalar.activation(
                        dct_sb[:, mbb, c * 128:(c + 1) * 128], pt[:], COPY
                    )
                if mbb == 1:
                    if c % 2 == 0:
                        nc.vector.tensor_scalar_mul(
                            dctn_sb[:, c * 128:(c + 1) * 128], pt[:], -1.0
                        )
                    else:
                        nc.scalar.activation(
                            dctn_sb[:, c * 128:(c + 1) * 128], pt[:], COPY,
                            scale=-1.0,
                        )

        # ---- Q/K projections ----------------------------------------------
        # Each parity phase runs as TWO chunk-major passes over 3 head-pair
        # groups (instead of one pass over 6):  the second pass refills the
        # chase bubbles left by the DMA pacing, frees a PSUM bank (psA=5,
        # psE=3 -> the exp chain never waits on the psE ping-pong), and for
        # K lets the first head-pairs evict one pass earlier, which starts
        # the serial exp chain sooner.
        def proj_pass(tiles, par, hps, tagn, after_chunk=None):
            groups = {
                hp: psA.tile([128, 128], F32, tag="A", name=f"g{tagn}{hp}")
                for hp in hps
            }
            for c in range(NCH):
                for hp in hps:
                    nc.tensor.matmul(
                        groups[hp][:],
                        lhsT=xc(tiles, par * NCH + c)[:, hp * 128:(hp + 1) * 128],
                        rhs=dctT_sb[:, c, par * 128:(par + 1) * 128],
                        start=(c == 0),
                        stop=(c == NCH - 1),
                    )
                if after_chunk:
                    after_chunk(c)
            return groups

        def evict_groups(groups, dst_sb, par, engs):
            for (hp, g), e in zip(sorted(groups.items()), engs):
                dst = dst_sb[:, hp, par * 128:(par + 1) * 128]
                if e == "v":
                    nc.vector.tensor_copy(dst, g[:])
                else:
                    nc.scalar.activation(dst, g[:], COPY)

        def evict_groups_bdq(groups, par, engs):
            # group (hp, par) [128 d-pair, 128 m] -> the two diagonal slots
            for (hp, g), e in zip(sorted(groups.items()), engs):
                for i in range(2):
                    dst = bdq[hp][i * 64:(i + 1) * 64, i, par * 128:(par + 1) * 128]
                    src = g[i * 64:(i + 1) * 64, :]
                    if e == "v":
                        nc.vector.tensor_copy(dst, src)
                    else:
                        nc.scalar.activation(dst, src, COPY)

        LO, HI = (0, 1, 2), (3, 4, 5)
        for par in range(MB):
            for hps in (LO, HI):
                g = proj_pass(q_t, par, hps, f"q{par}")
                evict_groups_bdq(g, par, "vsv" if hps is LO else "svs")

        # ---- energy helper: one head-PAIR x one k-block (1 mm + 1 exp) ----
        # E^T(pair)[kb-block, (head, m)] = kdT(kb-block)^T @ bdq: the zero
        # off-diagonal blocks keep the heads separate.
        p_tiles = [None] * HP   # per pair: [128, MB(kb), 2(head), M]

        def emit_pair(hp, kb):
            if p_tiles[hp] is None:
                p_tiles[hp] = pbuf.tile(
                    [128, MB, 2, M], BF, tag=f"p{hp}", name=f"p{hp}"
                )
            pe = psE.tile([128, 2, M], F32, tag="E", name=f"e{hp}{kb}")
            nc.tensor.matmul(
                pe[:],
                lhsT=kdT_sb[:, hp, kb * 128:(kb + 1) * 128],
                rhs=bdq[hp][:],
                start=True,
                stop=True,
            )
            nc.scalar.activation(
                p_tiles[hp][:, kb, :, :], pe[:], EXP, bias=ebias[:]
            )

        # interleave helper: emit energy pairs at odd chunks of a pass
        def pair_seq(pairs, kb):
            def after_chunk(c):
                if c % 2 == 1 and pairs:
                    emit_pair(pairs.pop(0), kb)
            return after_chunk

        # ---- K projections, energies paced into the pass bubbles ----------
        g = proj_pass(k_t, 0, LO, "k0")
        evict_groups(g, kdT_sb, 0, "vvs")   # gates the first energies
        g = proj_pass(k_t, 0, HI, "k0h", pair_seq([0, 1, 2], 0))
        evict_groups(g, kdT_sb, 0, "vvv")   # ACT is running the exp chain
        g = proj_pass(k_t, 1, LO, "k1", pair_seq([3, 4, 5], 0))
        evict_groups(g, kdT_sb, 1, "vvv")
        g = proj_pass(k_t, 1, HI, "k1h", pair_seq([0, 1, 2], 1))
        evict_groups(g, kdT_sb, 1, "vvv")

        # ---- V-proj: one fs column-half at a time --------------------------
        # fs0 sweep (heads 0-5) interleaves the last kb=1 energies; its Vd
        # rows land while the fs1 half of v is still streaming.
        nhp = FW // D  # 6 heads per split
        e1 = [3, 4, 5]

        def v_chunks(fs, vg, cs):
            for c in cs:               # c<8: A chunks; c>=8: B chunks
                par = c // NCH
                nc.tensor.matmul(
                    vg[par][:],
                    lhsT=dctT_sb[:, c % NCH, par * 128:(par + 1) * 128],
                    rhs=v_t[fs][c // QT][:, c % QT, :],
                    start=(c % NCH == 0),
                    stop=(c % NCH == NCH - 1),
                )
                if fs == 0 and c % 2 == 1 and e1:
                    emit_pair(e1.pop(0), 1)
                if c % NCH == NCH - 1:  # evict this parity's Vd rows
                    src = vg[par][:].rearrange("p (h x) -> p h x", x=D)
                    dst = vd_sb[:, par, fs * nhp:(fs + 1) * nhp, 0:D]
                    # last fs1 evict on ACT (idle after the exp chain) so
                    # Vector is free for the phase-B recip chain
                    if fs == 1 and par == 1:
                        nc.scalar.activation(dst, src, COPY)
                    else:
                        nc.vector.tensor_copy(dst, src)

        vg0 = [
            psA.tile([128, FW], F32, tag="A", name=f"vg0{par}")
            for par in range(MB)
        ]
        v_chunks(0, vg0, range(2 * NCH))

        # ---- phase B: ctx = P @ [Vd | 1] then normalize by the sums col ---
        def phase_b(h):
            hp, hi = h // 2, h % 2
            p_t = p_tiles[hp]
            # psE pool (idle after the exp chain): frees psA for the fs1 V
            # groups and the inverse-DCT rotation
            pc = psE.tile([128, MB, VW], F32, tag="E", name=f"c{h}")
            for mb in range(MB):
                for kb in range(MB):
                    nc.tensor.matmul(
                        pc[:, mb, :],
                        lhsT=p_t[:, kb, hi, mb * 128:(mb + 1) * 128],
                        rhs=vd_sb[:, kb, h, :],
                        start=(kb == 0),
                        stop=(kb == MB - 1),
                    )
            rs = rbuf.tile([128, MB], F32, tag="r", name=f"r{h}")
            nc.vector.reciprocal(rs[:], pc[:, :, D])
            for mb in range(MB):
                dst = ctx_sb[:, mb, h * D:(h + 1) * D]
                if (h + mb) % 2 == 0:
                    nc.vector.tensor_scalar_mul(
                        dst, pc[:, mb, 0:D], rs[:, mb:mb + 1]
                    )
                else:
                    nc.scalar.activation(
                        dst, pc[:, mb, 0:D], COPY, scale=rs[:, mb:mb + 1]
                    )

        # ---- inverse DCT --------------------------------------------------
        # half 0: x rows nb*128..      accumulate  dct_even@ctx_e + dct_odd@ctx_o
        # half 1: y rows NH+nb*128..   accumulate  dct_even@ctx_e + dctn @ctx_o
        # (host un-reverses the upper rows; the PE does the +- for free in
        # PSUM, copies evict alternating Vector/Scalar)
        def stage3_half(fs, extra=(), skip=0):
            extra = list(extra)
            NB = NH // 128  # 8 row-blocks per half
            # out-DMA pieces; the final piece is small so its transfer
            # starts (and finishes) sooner after the last eviction
            pieces = [(0, 4), (4, 4)] if fs == 0 else [(0, 4), (4, 2), (6, 2)]
            fslice = slice(fs * FW, (fs + 1) * FW)
            slot = 0
            for half in range(2):
                for p0, plen in pieces:
                    ost = ostage.tile(
                        [128, plen, FW], BF,
                        tag=f"o{fs}{half}{p0}", name=f"o{fs}{half}{p0}",
                    )
                    for nbi in range(plen):
                        nb = p0 + nbi
                        px = psA.tile(
                            [128, FW], F32, tag="A", name=f"x{fs}{half}{nb}"
                        )
                        nc.tensor.matmul(
                            px[:],
                            lhsT=dct_sb[:, 0, nb * 128:(nb + 1) * 128],
                            rhs=ctx_sb[:, 0, fslice],
                            start=True,
                            stop=False,
                        )
                        odd_lhsT = (
                            dct_sb[:, 1, nb * 128:(nb + 1) * 128] if half == 0
                            else dctn_sb[:, nb * 128:(nb + 1) * 128]
                        )
                        nc.tensor.matmul(
                            px[:],
                            lhsT=odd_lhsT,
                            rhs=ctx_sb[:, 1, fslice],
                            start=False,
                            stop=True,
                        )
                        if nb % 2 == 0:
                            nc.vector.tensor_copy(ost[:, nbi, :], px[:])
                        else:
                            nc.scalar.activation(ost[:, nbi, :], px[:], COPY)
                        slot += 1
                        if extra and slot > skip:
                            extra.pop(0)()
                    nc.sync.dma_start(
                        out_r[
                            fs,
                            :,
                            half * NB + p0:half * NB + p0 + plen,
                            :,
                        ],
                        ost[:],
                    )

        # phase B h0-5 interleaved with the fs1 V sweep: the fs1 v quarters
        # arrive exactly while the recip/normalize chains pace phase B, so
        # the fs1 projection rides in the PE bubbles.
        vg1 = [
            psA.tile([128, FW], F32, tag="A", name=f"vg1{par}")
            for par in range(MB)
        ]
        for h in range(6):   # heads 0..5 feed fs0
            phase_b(h)
            lo = h * 3 if h < 5 else 15
            v_chunks(1, vg1, range(lo, min(lo + 3, 16)) if h < 5 else [15])
        # interleave heads 6..11 into the fs0 sweep so the PE never waits on
        # the normalize chain
        extra = [(lambda hh=h: phase_b(hh)) for h in range(6, H)]
        stage3_half(0, extra, skip=2)
        stage3_half(1)

    nc.compile()
    return nc


def prep_in_maps(Q, K, V, mask, Q_dct):
    Q, K, V = np.asarray(Q), np.asarray(K), np.asarray(V)
    mask, Q_dct = np.asarray(mask), np.asarray(Q_dct)
    scale = np.float32(1.0 / np.sqrt(np.sqrt(np.float32(D))))
    m4 = mask.astype(np.float32)[:, None, :, None]        # [B,1,N,1]

    def fold(x):  # [B,N,HD] -> [A; B] along N
        lo, hi = x[:, :NH, :], x[:, NH:, :][:, ::-1, :]
        return np.concatenate([lo + hi, lo - hi], axis=1)

    def ptile(x, nch):  # [B, nch*128, F] -> [B, 128, nch, F] flat: row p*nch+c
        b, n, f = x.shape
        return x.reshape(b, nch, 128, f).transpose(0, 2, 1, 3).reshape(b, n, f)

    qs = fold((Q.astype(np.float32) * scale).transpose(0, 2, 1, 3).reshape(B, N, HD))
    ks = fold((K.astype(np.float32) * scale * m4).transpose(0, 2, 1, 3).reshape(B, N, HD))
    vs = fold((V.astype(np.float32) * m4).transpose(0, 2, 1, 3).reshape(B, N, HD))
    qs = np.ascontiguousarray(ptile(qs, 2 * NCH)).astype(_BF16)
    ks = np.ascontiguousarray(ptile(ks, 2 * NCH)).astype(_BF16)
    # v fs-major: [B, fs, p, c, FW] flattened to [B, 2N, FW]
    vs = np.stack(
        [ptile(vs[:, :, f * FW:(f + 1) * FW], 2 * NCH) for f in range(FSPLIT)],
        axis=1,
    ).reshape(B, 2 * N, FW)
    vs = np.ascontiguousarray(vs).astype(_BF16)

    dct_f = Q_dct.astype(np.float32)
    perm = np.concatenate([np.arange(0, M, 2), np.arange(1, M, 2)])
    dct_p = dct_f[perm]                            # rows reordered [even|odd]
    dctT = dct_p[:, :NH].T                         # [NH, M]
    dctT = np.ascontiguousarray(ptile(dctT[None], NCH)[0]).astype(_BF16)
    return [
        {"q": qs[b], "k": ks[b], "v": vs[b], "dctT": dctT}
        for b in range(B)
    ]


def unpack_out(out_arr):
    """Device 'out' [2*N, FW] (layout [fs, p, c-block, w]) -> [H, N, D]."""
    o = np.asarray(out_arr).astype(np.float32)
    o = o.reshape(FSPLIT, 128, 2 * NCH, FW)
    o = o.transpose(2, 1, 0, 3).reshape(N, HD)   # row c*128+p
    o[NH:] = o[NH:][::-1]   # upper rows hold y[j] = x[N-1-j]
    return o.reshape(N, H, D).transpose(1, 0, 2)


def run(Q, K, V, mask, Q_dct, trace=False):
    from concourse.bass_utils import run_bass_kernel_spmd

    if "nc" not in _CACHE:
        _CACHE["nc"] = build_nc()
    nc = _CACHE["nc"]
    in_maps = prep_in_maps(Q, K, V, mask, Q_dct)
    res = run_bass_kernel_spmd(nc, in_maps, core_ids=list(range(B)), trace=trace)
    x = np.stack([unpack_out(res.results[i]["out"]) for i in range(B)])
    return np.ascontiguousarray(x, dtype=np.float32), res


def kernel(Q, K, V, mask, Q_dct):
    x, _ = run(Q, K, V, mask, Q_dct, trace=False)
    return x
